# revision 5
# baseline (speedup 1.0000x reference)
"""AdaPT Linear (int8 systolic fake-quant matmul) on 8 TRN2 NeuronCores.

Reference semantics (single device):
    amax_x = max|x|, amax_w = max|w|         (global scalars)
    sx = 127/amax_x, sw = 127/amax_w
    qx = round(x*sx)  (int8), qw = round(w*sw)  (int8)
    out = (qx @ qw.T)_int32 / (sx*sw) + bias

Distribution: data-parallel over x rows (8 x 1024).  Each core:
  - computes partial amax over its x shard + its 512-row weight slice
  - tiny AllGather exchanges the 16 partial-amax scalars
  - quantizes x with an fp32 magic-number round (bit-exact round-half-even),
    stores int8-valued bf16, transposes via the XBAR DMA transpose
  - streams the full fp32 weight from HBM, transposes 128x128 tiles on the
    TensorEngine, quantizes on the way out of PSUM
  - bf16 matmul accumulates in fp32 PSUM: int8 products (<2^14) and sums
    (<2^24) are exact, so this reproduces the int8 systolic MAC bitwise
  - epilogue: out = psum * (1/(sx*sw)) + bias, DMA to DRAM

The whole thing is one NEFF; Tile generates all semaphores.
"""

import numpy as np

P = 128
MAGIC = 12582912.0  # 1.5 * 2**23: fp32 RNE round-to-int trick
MAXV = 127.0
NCORES = 8

# full-problem shapes (hardcoded per the task)
FULL_B, FULL_S, FULL_K = 4, 2048, 4096
FULL_N = 4096


def build_graph(M=1024, N=4096, K=4096, ncores=NCORES):
    """Build the SPMD Bass graph for one core (identical on all cores).

    M: x rows per core.  N: weight rows (= out cols).  K: contraction.
    """
    import concourse.bass as bass
    import concourse.mybir as mybir
    import concourse.tile as tile
    from concourse import bacc, bass_isa
    from concourse.masks import make_identity

    assert M % P == 0 and K % P == 0 and N % 512 == 0
    NSL = N // ncores      # weight slice rows per core (for amax)
    KT = K // P            # k tiles
    MB = M // P            # m blocks
    NB = N // 512          # n blocks of 512
    KC = min(2048, K)      # free-dim chunk for streaming f32
    KH = K // KC           # chunks per row-block of x / w

    f32 = mybir.dt.float32
    bf16 = mybir.dt.bfloat16

    nc = bacc.Bacc(None, num_devices=ncores)

    x_ext = nc.declare_dram_parameter("x", [M, K], f32, isOutput=False)
    w_ext = nc.declare_dram_parameter("w", [N, K], f32, isOutput=False)
    wsl_ext = nc.declare_dram_parameter("wslice", [NSL, K], f32, isOutput=False)
    b_ext = nc.declare_dram_parameter("bias", [N], f32, isOutput=False)
    out_ext = nc.declare_dram_parameter("out", [M, N], f32, isOutput=True)

    qx_dram = nc.dram_tensor("qx_scratch", [M, K], bf16)
    cc_in = nc.dram_tensor("cc_in", [1, 2], f32)
    cc_out = nc.dram_tensor("cc_out", [ncores, 2], f32)

    with tile.TileContext(nc) as tc:
        with (
            tc.tile_pool(name="xchunks", bufs=2) as xpool,
            tc.tile_pool(name="wchunks", bufs=2) as wpool,
            tc.tile_pool(name="qxc", bufs=2) as qxpool,
            tc.tile_pool(name="persist", bufs=1) as persist,
            tc.tile_pool(name="qwt", bufs=2) as qwtpool,
            tc.tile_pool(name="tw", bufs=2) as twpool,
            tc.tile_pool(name="ob", bufs=3) as obpool,
            tc.tile_pool(name="stats", bufs=1) as stats,
            tc.tile_pool(name="psum_tr", bufs=2, space="PSUM") as pstr,
            tc.tile_pool(name="psum_mm", bufs=2, space="PSUM") as psmm,
        ):
            # ---------------- Phase A: local amax ----------------
            xmaxes = stats.tile([P, MB * KH], f32)
            for i in range(MB):
                for h in range(KH):
                    xc = xpool.tile([P, KC], f32)
                    nc.sync.dma_start(out=xc, in_=x_ext[i * P:(i + 1) * P, h * KC:(h + 1) * KC])
                    nc.vector.tensor_reduce(
                        out=xmaxes[:, i * KH + h:i * KH + h + 1], in_=xc,
                        axis=mybir.AxisListType.X, op=mybir.AluOpType.max,
                        apply_absolute_value=True)
            wmaxes = stats.tile([P, (NSL // P) * KH], f32)
            for i in range(NSL // P):
                for h in range(KH):
                    wc = wpool.tile([P, KC], f32)
                    nc.sync.dma_start(out=wc, in_=wsl_ext[i * P:(i + 1) * P, h * KC:(h + 1) * KC])
                    nc.vector.tensor_reduce(
                        out=wmaxes[:, i * KH + h:i * KH + h + 1], in_=wc,
                        axis=mybir.AxisListType.X, op=mybir.AluOpType.max,
                        apply_absolute_value=True)
            xmax_v = stats.tile([P, 1], f32)
            wmax_v = stats.tile([P, 1], f32)
            nc.vector.tensor_reduce(out=xmax_v, in_=xmaxes, axis=mybir.AxisListType.X,
                                    op=mybir.AluOpType.max)
            nc.vector.tensor_reduce(out=wmax_v, in_=wmaxes, axis=mybir.AxisListType.X,
                                    op=mybir.AluOpType.max)
            # cross-partition max -> every partition holds the core-local amax
            xmax_p = stats.tile([P, 1], f32)
            wmax_p = stats.tile([P, 1], f32)
            nc.gpsimd.partition_all_reduce(xmax_p, xmax_v, channels=P,
                                           reduce_op=bass_isa.ReduceOp.max)
            nc.gpsimd.partition_all_reduce(wmax_p, wmax_v, channels=P,
                                           reduce_op=bass_isa.ReduceOp.max)

            # ---------------- Phase B: exchange + scales ----------------
            pack = stats.tile([1, 2], f32)
            nc.vector.tensor_copy(pack[0:1, 0:1], xmax_p[0:1, :])
            nc.vector.tensor_copy(pack[0:1, 1:2], wmax_p[0:1, :])
            nc.sync.dma_start(out=cc_in[:], in_=pack[:])
            nc.gpsimd.collective_compute(
                "AllGather", mybir.AluOpType.bypass,
                replica_groups=[list(range(ncores))],
                ins=[cc_in[:].opt()], outs=[cc_out[:].opt()])
            gat = stats.tile([ncores, 2], f32)
            nc.sync.dma_start(out=gat, in_=cc_out[:])
            gmax = stats.tile([ncores, 2], f32)
            nc.gpsimd.partition_all_reduce(gmax, gat, channels=ncores,
                                           reduce_op=bass_isa.ReduceOp.max)
            ax = gmax[0:1, 0:1]   # global amax_x
            aw = gmax[0:1, 1:2]   # global amax_w

            sc = stats.tile([1, 8], f32)  # scratch scalars
            sx_t = stats.tile([1, 1], f32)
            sw_t = stats.tile([1, 1], f32)
            ds_t = stats.tile([1, 1], f32)

            def recip(dst, src, t0, t1):
                # dst = 1/src with one Newton step on top of InstReciprocal
                nc.vector.reciprocal(dst, src)
                nc.vector.tensor_tensor(out=t0, in0=src, in1=dst,
                                        op=mybir.AluOpType.mult)
                nc.vector.tensor_scalar(out=t1, in0=t0, scalar1=-1.0, scalar2=2.0,
                                        op0=mybir.AluOpType.mult,
                                        op1=mybir.AluOpType.add)
                nc.vector.tensor_tensor(out=dst, in0=dst, in1=t1,
                                        op=mybir.AluOpType.mult)

            recip(sc[0:1, 0:1], ax, sc[0:1, 2:3], sc[0:1, 3:4])
            nc.vector.tensor_scalar(out=sx_t, in0=sc[0:1, 0:1], scalar1=MAXV,
                                    scalar2=None, op0=mybir.AluOpType.mult)
            recip(sc[0:1, 1:2], aw, sc[0:1, 2:3], sc[0:1, 3:4])
            nc.vector.tensor_scalar(out=sw_t, in0=sc[0:1, 1:2], scalar1=MAXV,
                                    scalar2=None, op0=mybir.AluOpType.mult)
            nc.vector.tensor_tensor(out=sc[0:1, 4:5], in0=sx_t, in1=sw_t,
                                    op=mybir.AluOpType.mult)
            recip(ds_t, sc[0:1, 4:5], sc[0:1, 5:6], sc[0:1, 6:7])

            sxb = stats.tile([P, 1], f32)
            swb = stats.tile([P, 1], f32)
            dsb = stats.tile([P, 1], f32)
            nc.gpsimd.partition_broadcast(sxb, sx_t)
            nc.gpsimd.partition_broadcast(swb, sw_t)
            nc.gpsimd.partition_broadcast(dsb, ds_t)

            # bias, replicated into all partitions (fp32)
            bias_t = persist.tile([P, N], f32)
            bias_bcast = bass.AP(tensor=b_ext, offset=0, ap=[[0, P], [1, N]])
            nc.sync.dma_start(out=bias_t, in_=bias_bcast)

            # identity for TensorE transposes
            ident = persist.tile([P, P], f32)
            make_identity(nc, ident[:])

            # ---------------- Phase C: quantize + transpose x ----------------
            for i in range(MB):
                for h in range(KH):
                    xc = xpool.tile([P, KC], f32)
                    nc.sync.dma_start(out=xc, in_=x_ext[i * P:(i + 1) * P, h * KC:(h + 1) * KC])
                    # t = x*sx + MAGIC  (fp32 RNE makes t = round(x*sx) + MAGIC)
                    nc.vector.tensor_scalar(out=xc, in0=xc, scalar1=sxb,
                                            scalar2=MAGIC, op0=mybir.AluOpType.mult,
                                            op1=mybir.AluOpType.add)
                    qc = qxpool.tile([P, KC], bf16)
                    nc.scalar.activation(out=qc, in_=xc,
                                         func=mybir.ActivationFunctionType.Copy,
                                         bias=-MAGIC, scale=1.0)
                    nc.sync.dma_start(out=qx_dram[i * P:(i + 1) * P, h * KC:(h + 1) * KC],
                                      in_=qc)
            # XBAR transpose: qxT[p, kt, m] = qx[m, kt*128+p]
            qxT = persist.tile([P, KT, M], bf16)
            MTR = min(M, 512)
            for kt in range(KT):
                for mh in range(M // MTR):
                    nc.sync.dma_start(
                        out=qxT[:, kt, mh * MTR:(mh + 1) * MTR],
                        in_=qx_dram[mh * MTR:(mh + 1) * MTR, kt * P:(kt + 1) * P],
                        transpose=True)

            # ---------------- Phase D: weight stream + matmul ----------------
            for nb in range(NB):
                qwT = qwtpool.tile([P, KT, 512], bf16)
                for s in range(4):          # 128-row sub-blocks of this n-block
                    n0 = nb * 512 + s * P
                    for h in range(KH):
                        wc = wpool.tile([P, KC], f32)
                        nc.sync.dma_start(out=wc, in_=w_ext[n0:n0 + P, h * KC:(h + 1) * KC])
                        for g in range(KC // 512):   # groups of 4 k-tiles
                            ps = pstr.tile([P, 4, P], f32, space="PSUM")
                            for j in range(4):
                                ktl = g * 4 + j
                                nc.tensor.transpose(
                                    ps[:, j, :], wc[:, ktl * P:(ktl + 1) * P], ident[:])
                            twt = twpool.tile([P, 4, P], f32)
                            nc.scalar.activation(out=twt, in_=ps,
                                                 func=mybir.ActivationFunctionType.Copy,
                                                 bias=MAGIC, scale=swb)
                            kt0 = h * (KC // P) + g * 4
                            nc.gpsimd.tensor_scalar(
                                out=qwT[:, kt0:kt0 + 4, s * P:(s + 1) * P],
                                in0=twt, scalar1=-MAGIC, scalar2=None,
                                op0=mybir.AluOpType.add)
                for mb in range(MB):
                    acc = psmm.tile([P, 512], f32, space="PSUM")
                    for kt in range(KT):
                        nc.tensor.matmul(
                            acc, qxT[:, kt, mb * P:(mb + 1) * P], qwT[:, kt, :],
                            start=(kt == 0), stop=(kt == KT - 1))
                    ob = obpool.tile([P, 512], f32)
                    nc.vector.scalar_tensor_tensor(
                        out=ob, in0=acc, scalar=dsb,
                        in1=bias_t[:, nb * 512:(nb + 1) * 512],
                        op0=mybir.AluOpType.mult, op1=mybir.AluOpType.add)
                    nc.sync.dma_start(
                        out=out_ext[mb * P:(mb + 1) * P, nb * 512:(nb + 1) * 512],
                        in_=ob)
    nc.compile()
    return nc


def shard_inputs(x, weight, bias, M=1024, N=4096, ncores=NCORES):
    xf = np.ascontiguousarray(np.asarray(x, dtype=np.float32).reshape(-1, x.shape[-1]))
    w = np.ascontiguousarray(np.asarray(weight, dtype=np.float32))
    b = np.ascontiguousarray(np.asarray(bias, dtype=np.float32))
    nsl = N // ncores
    in_maps = []
    for c in range(ncores):
        in_maps.append({
            "x": np.ascontiguousarray(xf[c * M:(c + 1) * M]),
            "w": w,
            "wslice": np.ascontiguousarray(w[c * nsl:(c + 1) * nsl]),
            "bias": b,
        })
    return in_maps


def _run(x, weight, bias, trace=False):
    from concourse.bass_utils import run_bass_kernel_spmd

    nc = build_graph()
    in_maps = shard_inputs(x, weight, bias)
    res = run_bass_kernel_spmd(nc, in_maps, core_ids=list(range(NCORES)),
                               trace=trace)
    outs = [res.results[c]["out"] for c in range(NCORES)]
    full = np.concatenate(outs, axis=0).reshape(FULL_B, FULL_S, FULL_N)
    return full.astype(np.float32), res


def kernel(x, weight, bias):
    out, _ = _run(x, weight, bias, trace=False)
    return out


# revision 8
# speedup vs baseline: 2.5860x; 2.5860x over previous
"""AdaPT Linear (int8 systolic fake-quant matmul) on 8 TRN2 NeuronCores.

Reference semantics (single device):
    amax_x = max|x|, amax_w = max|w|         (global scalars)
    sx = 127/amax_x, sw = 127/amax_w
    qx = round(x*sx)  (int8), qw = round(w*sw)  (int8)
    out = (qx @ qw.T)_int32 / (sx*sw) + bias

Distribution: data-parallel over x rows (8 x 1024).  Each core:
  - computes partial amax over its x shard + its 512-row weight slice
  - two tiny AllGathers exchange the partial-amax scalars (amax_w first so
    the weight pipeline unblocks early)
  - quantizes x with an fp32 magic-number round (bit-exact round-half-even),
    stores int8-valued bf16, transposes via the XBAR DMA transpose
  - streams the full fp32 weight from HBM, transposes 128x128 tiles on the
    TensorEngine (f32r: 1.5 cyc/row), quantizes on the way out of PSUM
    (ACT pass1 + DVE pass2)
  - bf16 matmul accumulates in fp32 PSUM: int8 products (<2^14) and sums
    (<2^24) are exact, so this reproduces the int8 systolic MAC
  - epilogue: out = psum * (1/(sx*sw)) + bias (one DVE op), DMA to DRAM

The whole thing is one NEFF; Tile generates all semaphores.
"""

import numpy as np

P = 128
MAGIC = 12582912.0  # 1.5 * 2**23: fp32 RNE round-to-int trick
MAXV = 127.0
NCORES = 8

# full-problem shapes (hardcoded per the task)
FULL_B, FULL_S, FULL_K = 4, 2048, 4096
FULL_N = 4096


def build_graph(M=1024, N=4096, K=4096, ncores=NCORES):
    """Build the SPMD Bass graph for one core (identical on all cores)."""
    import concourse.bass as bass
    import concourse.mybir as mybir
    import concourse.tile as tile
    from concourse import bacc, bass_isa
    from concourse.masks import make_identity

    assert M % P == 0 and K % P == 0 and N % 512 == 0
    NSL = N // ncores      # weight slice rows per core (for amax)
    KT = K // P            # k tiles
    MB = M // P            # m blocks
    NB = N // 512          # n blocks of 512
    KC = min(2048, K)      # free-dim chunk for streaming f32
    KH = K // KC           # chunks per row-block of x / w
    G = min(8, KC // P)    # k-tiles per transpose/quant group

    f32 = mybir.dt.float32
    f32r = mybir.dt.float32r
    bf16 = mybir.dt.bfloat16

    nc = bacc.Bacc(None, num_devices=ncores)

    x_ext = nc.declare_dram_parameter("x", [M, K], f32, isOutput=False)
    w_ext = nc.declare_dram_parameter("w", [N, K], f32, isOutput=False)
    wsl_ext = nc.declare_dram_parameter("wslice", [NSL, K], f32, isOutput=False)
    b_ext = nc.declare_dram_parameter("bias", [N], f32, isOutput=False)
    out_ext = nc.declare_dram_parameter("out", [M, N], f32, isOutput=True)

    qx_dram = nc.dram_tensor("qx_scratch", [M, K], bf16)
    ccw_in = nc.dram_tensor("ccw_in", [1, 1], f32)
    ccw_out = nc.dram_tensor("ccw_out", [ncores, 1], f32)
    ccx_in = nc.dram_tensor("ccx_in", [1, 1], f32)
    ccx_out = nc.dram_tensor("ccx_out", [ncores, 1], f32)

    with tile.TileContext(nc) as tc:
        with (
            tc.tile_pool(name="xchunks", bufs=2) as xpool,
            tc.tile_pool(name="wchunks", bufs=2) as wpool,
            tc.tile_pool(name="qxc", bufs=2) as qxpool,
            tc.tile_pool(name="persist", bufs=1) as persist,
            tc.tile_pool(name="qwt", bufs=2) as qwtpool,
            tc.tile_pool(name="tw", bufs=2) as twpool,
            tc.tile_pool(name="ob", bufs=3) as obpool,
            tc.tile_pool(name="stats", bufs=1) as stats,
            tc.tile_pool(name="psum_tr", bufs=2, space="PSUM") as pstr,
            tc.tile_pool(name="psum_mm", bufs=2, space="PSUM") as psmm,
        ):
            rg = [list(range(ncores))]

            # ---------- Phase A1: weight-slice amax + exchange ----------
            wmaxes = stats.tile([P, (NSL // P) * KH], f32)
            for i in range(NSL // P):
                for h in range(KH):
                    wc = wpool.tile([P, KC], f32)
                    nc.sync.dma_start(out=wc, in_=wsl_ext[i * P:(i + 1) * P, h * KC:(h + 1) * KC])
                    nc.vector.tensor_reduce(
                        out=wmaxes[:, i * KH + h:i * KH + h + 1], in_=wc,
                        axis=mybir.AxisListType.X, op=mybir.AluOpType.max,
                        apply_absolute_value=True)
            wmax_v = stats.tile([P, 1], f32)
            nc.vector.tensor_reduce(out=wmax_v, in_=wmaxes, axis=mybir.AxisListType.X,
                                    op=mybir.AluOpType.max)
            wmax_p = stats.tile([P, 1], f32)
            nc.gpsimd.partition_all_reduce(wmax_p, wmax_v, channels=P,
                                           reduce_op=bass_isa.ReduceOp.max)
            nc.sync.dma_start(out=ccw_in[:], in_=wmax_p[0:1, :])
            nc.gpsimd.collective_compute(
                "AllGather", mybir.AluOpType.bypass, replica_groups=rg,
                ins=[ccw_in[:].opt()], outs=[ccw_out[:].opt()])
            gat_w = stats.tile([ncores, 1], f32)
            nc.sync.dma_start(out=gat_w, in_=ccw_out[:])
            gmax_w = stats.tile([ncores, 1], f32)
            nc.gpsimd.partition_all_reduce(gmax_w, gat_w, channels=ncores,
                                           reduce_op=bass_isa.ReduceOp.max)
            aw = gmax_w[0:1, 0:1]

            # ---------- Phase A2: x amax + exchange ----------
            xmaxes = stats.tile([P, MB * KH], f32)
            for i in range(MB):
                for h in range(KH):
                    xc = xpool.tile([P, KC], f32)
                    nc.sync.dma_start(out=xc, in_=x_ext[i * P:(i + 1) * P, h * KC:(h + 1) * KC])
                    nc.vector.tensor_reduce(
                        out=xmaxes[:, i * KH + h:i * KH + h + 1], in_=xc,
                        axis=mybir.AxisListType.X, op=mybir.AluOpType.max,
                        apply_absolute_value=True)
            xmax_v = stats.tile([P, 1], f32)
            nc.vector.tensor_reduce(out=xmax_v, in_=xmaxes, axis=mybir.AxisListType.X,
                                    op=mybir.AluOpType.max)
            xmax_p = stats.tile([P, 1], f32)
            nc.gpsimd.partition_all_reduce(xmax_p, xmax_v, channels=P,
                                           reduce_op=bass_isa.ReduceOp.max)
            nc.sync.dma_start(out=ccx_in[:], in_=xmax_p[0:1, :])
            nc.gpsimd.collective_compute(
                "AllGather", mybir.AluOpType.bypass, replica_groups=rg,
                ins=[ccx_in[:].opt()], outs=[ccx_out[:].opt()])
            gat_x = stats.tile([ncores, 1], f32)
            nc.sync.dma_start(out=gat_x, in_=ccx_out[:])
            gmax_x = stats.tile([ncores, 1], f32)
            nc.gpsimd.partition_all_reduce(gmax_x, gat_x, channels=ncores,
                                           reduce_op=bass_isa.ReduceOp.max)
            ax = gmax_x[0:1, 0:1]

            # ---------- scales ----------
            scw = stats.tile([1, 4], f32)
            scx = stats.tile([1, 4], f32)
            sx_t = stats.tile([1, 1], f32)
            sw_t = stats.tile([1, 1], f32)
            ds_t = stats.tile([1, 1], f32)
            dsc = stats.tile([1, 4], f32)

            def recip(dst, src, t0, t1):
                nc.vector.reciprocal(dst, src)
                nc.vector.tensor_tensor(out=t0, in0=src, in1=dst,
                                        op=mybir.AluOpType.mult)
                nc.vector.tensor_scalar(out=t1, in0=t0, scalar1=-1.0, scalar2=2.0,
                                        op0=mybir.AluOpType.mult,
                                        op1=mybir.AluOpType.add)
                nc.vector.tensor_tensor(out=dst, in0=dst, in1=t1,
                                        op=mybir.AluOpType.mult)

            recip(scw[0:1, 0:1], aw, scw[0:1, 1:2], scw[0:1, 2:3])
            nc.vector.tensor_scalar(out=sw_t, in0=scw[0:1, 0:1], scalar1=MAXV,
                                    scalar2=None, op0=mybir.AluOpType.mult)
            swb = stats.tile([P, 1], f32)
            nc.gpsimd.partition_broadcast(swb, sw_t)

            recip(scx[0:1, 0:1], ax, scx[0:1, 1:2], scx[0:1, 2:3])
            nc.vector.tensor_scalar(out=sx_t, in0=scx[0:1, 0:1], scalar1=MAXV,
                                    scalar2=None, op0=mybir.AluOpType.mult)
            sxb = stats.tile([P, 1], f32)
            nc.gpsimd.partition_broadcast(sxb, sx_t)

            nc.vector.tensor_tensor(out=dsc[0:1, 0:1], in0=sx_t, in1=sw_t,
                                    op=mybir.AluOpType.mult)
            recip(ds_t, dsc[0:1, 0:1], dsc[0:1, 1:2], dsc[0:1, 2:3])
            dsb = stats.tile([P, 1], f32)
            nc.gpsimd.partition_broadcast(dsb, ds_t)

            # bias replicated into all partitions (fp32)
            bias_t = persist.tile([P, N], f32)
            bias_bcast = bass.AP(tensor=b_ext, offset=0, ap=[[0, P], [1, N]])
            nc.sync.dma_start(out=bias_t, in_=bias_bcast)

            # identity for TensorE transposes
            ident = persist.tile([P, P], f32)
            make_identity(nc, ident[:])

            # ---------- Phase C: quantize + transpose x ----------
            for i in range(MB):
                for h in range(KH):
                    xc = xpool.tile([P, KC], f32)
                    nc.sync.dma_start(out=xc, in_=x_ext[i * P:(i + 1) * P, h * KC:(h + 1) * KC])
                    nc.vector.tensor_scalar(out=xc, in0=xc, scalar1=sxb,
                                            scalar2=MAGIC, op0=mybir.AluOpType.mult,
                                            op1=mybir.AluOpType.add)
                    qc = qxpool.tile([P, KC], bf16)
                    nc.scalar.activation(out=qc, in_=xc,
                                         func=mybir.ActivationFunctionType.Copy,
                                         bias=-MAGIC, scale=1.0)
                    nc.sync.dma_start(out=qx_dram[i * P:(i + 1) * P, h * KC:(h + 1) * KC],
                                      in_=qc)
            # XBAR transpose: qxT[p, kt, m] = qx[m, kt*128+p]
            qxT = persist.tile([P, KT, M], bf16)
            MTR = min(M, 512)
            for kt in range(KT):
                for mh in range(M // MTR):
                    nc.sync.dma_start(
                        out=qxT[:, kt, mh * MTR:(mh + 1) * MTR],
                        in_=qx_dram[mh * MTR:(mh + 1) * MTR, kt * P:(kt + 1) * P],
                        transpose=True)

            # ---------- Phase D: weight stream + matmul ----------
            for nb in range(NB):
                qwT = qwtpool.tile([P, KT, 512], bf16)
                for s in range(4):          # 128-row sub-blocks of this n-block
                    n0 = nb * 512 + s * P
                    for h in range(KH):
                        wc = wpool.tile([P, KC], f32)
                        nc.sync.dma_start(out=wc, in_=w_ext[n0:n0 + P, h * KC:(h + 1) * KC])
                        for g in range(KC // (G * P)):   # groups of G k-tiles
                            ps = pstr.tile([P, G, P], f32, space="PSUM")
                            for j in range(G):
                                ktl = g * G + j
                                nc.tensor.transpose(
                                    ps[:, j, :], wc[:, ktl * P:(ktl + 1) * P],
                                    ident[:])
                            twt = twpool.tile([P, G, P], f32)
                            nc.scalar.activation(out=twt, in_=ps,
                                                 func=mybir.ActivationFunctionType.Copy,
                                                 bias=MAGIC, scale=swb)
                            kt0 = h * (KC // P) + g * G
                            nc.vector.tensor_scalar(
                                out=qwT[:, kt0:kt0 + G, s * P:(s + 1) * P],
                                in0=twt, scalar1=-MAGIC, scalar2=None,
                                op0=mybir.AluOpType.add)
                for mb in range(MB):
                    acc = psmm.tile([P, 512], f32, space="PSUM")
                    for kt in range(KT):
                        nc.tensor.matmul(
                            acc, qxT[:, kt, mb * P:(mb + 1) * P], qwT[:, kt, :],
                            start=(kt == 0), stop=(kt == KT - 1))
                    ob = obpool.tile([P, 512], f32)
                    nc.vector.scalar_tensor_tensor(
                        out=ob, in0=acc, scalar=dsb,
                        in1=bias_t[:, nb * 512:(nb + 1) * 512],
                        op0=mybir.AluOpType.mult, op1=mybir.AluOpType.add)
                    nc.sync.dma_start(
                        out=out_ext[mb * P:(mb + 1) * P, nb * 512:(nb + 1) * 512],
                        in_=ob)
    nc.compile()
    return nc


def shard_inputs(x, weight, bias, M=1024, N=4096, ncores=NCORES):
    xf = np.ascontiguousarray(np.asarray(x, dtype=np.float32).reshape(-1, x.shape[-1]))
    w = np.ascontiguousarray(np.asarray(weight, dtype=np.float32))
    b = np.ascontiguousarray(np.asarray(bias, dtype=np.float32))
    nsl = N // ncores
    in_maps = []
    for c in range(ncores):
        in_maps.append({
            "x": np.ascontiguousarray(xf[c * M:(c + 1) * M]),
            "w": w,
            "wslice": np.ascontiguousarray(w[c * nsl:(c + 1) * nsl]),
            "bias": b,
        })
    return in_maps


def _run(x, weight, bias, trace=False):
    from concourse.bass_utils import run_bass_kernel_spmd

    nc = build_graph()
    in_maps = shard_inputs(x, weight, bias)
    res = run_bass_kernel_spmd(nc, in_maps, core_ids=list(range(NCORES)),
                               trace=trace)
    outs = [res.results[c]["out"] for c in range(NCORES)]
    full = np.concatenate(outs, axis=0).reshape(FULL_B, FULL_S, FULL_N)
    return full.astype(np.float32), res


def kernel(x, weight, bias):
    out, _ = _run(x, weight, bias, trace=False)
    return out
